# revision 31
# baseline (speedup 1.0000x reference)
"""CrossAttentionBlock Trainium2 kernel (8 NeuronCores), v2.

Sharding: 2-way data parallel over batch x 4-way tensor parallel
(attention: 3 of 12 heads per core; MLP: 512 of 2048 tokens per core
after a ReduceScatter of the o-projection partials).

v2 changes vs baseline:
  - LN affine (w,b) folded into the following projection weights
    host-side (exact); LN on device is just (x-m)*rsqrt(var+eps).
  - Two-pass LN with a single batched reciprocal (DVE reciprocal has
    ~1us fixed cost per call).
  - RoPE applied to 4 token tiles per call (6 big DVE ops instead of 24
    small ones per group).
  - Scores: heads 0,1 issued as adjacent matmuls on disjoint PE row
    groups (concurrent); head 2 split (kv tiles i / i+8) across row
    groups with q/k head-2 data duplicated into partitions 64-127.
  - Softmax exp reads score PSUM directly (no DVE staging copies);
    probs written as fp8 for the DoubleRow attn*V matmuls.
  - Softmax denominators: v_aug carries a 64-wide ones block so the
    attn PSUM rows 64-127 hold the denominator; a 1-row bf16 copy +
    K=1 broadcast matmul + single DVE divide normalizes.
  - o-projection: head-2 (K=64) matmuls packed pairwise onto disjoint
    row groups (wo2/at2 duplicated to partitions 64-127).
  - MLP in fp8 DoubleRow (weights x64 host-side; 1/64 folded into the
    GELU / output activation scale).
  - ReduceScatter in bf16, split in two to overlap with o-proj tail.
"""

import numpy as np
import ml_dtypes

import concourse.bass as bass
import concourse.tile as tile
import concourse.mybir as mybir

BF = mybir.dt.bfloat16
E4 = mybir.dt.float8e4
F32 = mybir.dt.float32
I32 = mybir.dt.int32
AF = mybir.ActivationFunctionType
ALU = mybir.AluOpType
PM = mybir.MatmulPerfMode

DIM = 768
H = 12
DH = 64
MLP_H = 3072
B = 2
P = 2048          # both PQ and PKV
N_CORES = 8
G = 4             # cores per batch group
HPC = H // G      # heads per core = 3
TPC = P // G      # tokens per core for MLP = 512
ROPE_THETA = 100.0
LN_EPS = 1e-5
GROUPS = [[0, 1, 2, 3], [4, 5, 6, 7]]
GROUPS8 = [[0, 1, 2, 3, 4, 5, 6, 7]]
W8 = 64.0         # fp8 weight scale for the MLP

NT = P // 128     # 16 token tiles
KT = DIM // 128   # 6 feature tiles
QC = 512          # q chunk in attention
NQC = P // QC     # 4


def _split_multi_waits(nc, max_waits: int = 1):
    """Walrus codegen in this container accepts at most one sync wait per
    instruction; Tile's post-scheduler drain/barrier can carry more. Move
    the excess onto same-engine nops inserted just before."""
    for bb in nc.main_func.blocks:
        i = 0
        insts = bb.instructions
        while i < len(insts):
            ins = insts[i]
            si = ins.sync_info
            if si is not None and si.on_wait and len(si.on_wait) > max_waits:
                waits = list(si.on_wait)
                keep = waits[-max_waits:]
                extra = waits[:-max_waits]
                nops = []
                for w in extra:
                    nop = mybir.InstNoOp(
                        name=f"I-waitfix-{nc.next_id()}", engine=ins.engine
                    )
                    nop.sync_info = mybir.SyncInfo(on_wait=[w], on_update=[])
                    nops.append(nop)
                ins.sync_info = mybir.SyncInfo(
                    on_wait=keep, on_update=list(si.on_update or [])
                )
                for j, nop in enumerate(nops):
                    insts.insert(i + j, nop)
                i += len(nops)
            i += 1


def build_nc():
    nc = bass.Bass("TRN2", target_bir_lowering=False, debug=False,
                   num_devices=N_CORES)

    # ---------------- inputs ----------------
    query = nc.dram_tensor("query", [P, DIM], F32, kind="ExternalInput")
    kv = nc.dram_tensor("kv", [P, DIM], F32, kind="ExternalInput")
    q_res_t = nc.dram_tensor("q_res_t", [DIM, TPC], F32, kind="ExternalInput")
    msk = nc.dram_tensor("msk", [128, 2], F32, kind="ExternalInput")
    posq = nc.dram_tensor("posq", [128, NT, 2], I32, kind="ExternalInput")
    poskv = nc.dram_tensor("poskv", [128, NT, 2], I32, kind="ExternalInput")
    trig = nc.dram_tensor("trig", [64, 32], F32, kind="ExternalInput")
    wq_s = nc.dram_tensor("wq_s", [DIM, HPC * DH], BF, kind="ExternalInput")
    wkv_s = nc.dram_tensor("wkv_s", [DIM, 2 * HPC * DH], BF, kind="ExternalInput")
    bqkv_s = nc.dram_tensor("bqkv_s", [3 * HPC * DH], F32, kind="ExternalInput")
    wo_a = nc.dram_tensor("wo_a", [G * 128, DIM], BF, kind="ExternalInput")
    wo_b = nc.dram_tensor("wo_b", [G * 64, DIM], BF, kind="ExternalInput")
    w1 = nc.dram_tensor("w1", [DIM, MLP_H], E4, kind="ExternalInput")
    b1 = nc.dram_tensor("b1", [MLP_H], F32, kind="ExternalInput")
    w2 = nc.dram_tensor("w2", [MLP_H, DIM], E4, kind="ExternalInput")
    b2 = nc.dram_tensor("b2", [DIM], F32, kind="ExternalInput")
    out_t = nc.dram_tensor("out_t", [DIM, TPC], F32, kind="ExternalOutput")

    def bcast_ap(t, n_part, free):
        return bass.AP(tensor=t.ap().tensor, offset=0,
                       ap=[[0, n_part], [1, free]])

    with tile.TileContext(nc) as tc:
        with (
            tc.tile_pool(name="consts", bufs=1) as consts,
            tc.tile_pool(name="mlpw", bufs=1) as mlpw,
            tc.tile_pool(name="xP", bufs=1) as xP,
            tc.tile_pool(name="work", bufs=3) as work,
            tc.tile_pool(name="dram", bufs=1, space="DRAM") as dram,
        ):
            # ---------------- constants ----------------
            ones_bf = consts.tile([128, 128], BF)
            nc.vector.memset(ones_bf[:], 1.0)
            ones_f32 = consts.tile([128, 128], F32)
            nc.vector.memset(ones_f32[:], 1.0)
            eps_sb = consts.tile([128, 1], F32)
            nc.vector.memset(eps_sb[:], LN_EPS)
            b1_sb = consts.tile([128, MLP_H // 128], F32)
            nc.sync.dma_start(b1_sb[:], b1.rearrange("(m p) -> p m", p=128))
            b2_sb = consts.tile([128, KT], F32)
            nc.sync.dma_start(b2_sb[:], b2.rearrange("(m p) -> p m", p=128))
            msk_sb = consts.tile([128, 2], F32)
            nc.sync.dma_start(msk_sb[:], msk[:])
            posq_sb = consts.tile([128, NT, 2], I32)
            nc.sync.dma_start(posq_sb[:], posq[:])
            poskv_sb = consts.tile([128, NT, 2], I32)
            nc.sync.dma_start(poskv_sb[:], poskv[:])

            qn_dram = dram.tile([P, DIM], BF)
            kvn_dram = dram.tile([P, DIM], BF)
            qrot_dram = dram.tile([P, 256], BF)
            krot_dram = dram.tile([P, 256], BF)
            ag_a_in = dram.tile([2 * G, 128, TPC], BF)
            ag_a_out = dram.tile([2 * G, 128, TPC], BF)
            ag_b_in = dram.tile([2 * G, 64, TPC], BF)
            ag_b_out = dram.tile([2 * G, 64, TPC], BF)

            with tc.tile_pool(name="attnP", bufs=1) as attnP:
                # attention-lifetime tiles
                wq_sb = attnP.tile([128, KT, HPC * DH], BF)
                nc.sync.dma_start(wq_sb[:],
                                  wq_s.rearrange("(k p) n -> p k n", p=128))
                wkv_sb = attnP.tile([128, KT, 2 * HPC * DH], BF)
                nc.sync.dma_start(wkv_sb[:],
                                  wkv_s.rearrange("(k p) n -> p k n", p=128))
                woa_sb = attnP.tile([128, G, DIM], BF)
                nc.sync.dma_start(woa_sb[:],
                                  wo_a.rearrange("(k p) n -> p k n", p=128))
                wob_sb = attnP.tile([128, 2, DIM], BF)
                nc.sync.dma_start(wob_sb[:],
                                  wo_b.rearrange("(k p) n -> p k n", p=128))
                bqkv_rep = attnP.tile([128, 3 * HPC * DH], F32)
                nc.sync.dma_start(bqkv_rep[:],
                                  bcast_ap(bqkv_s, 128, 3 * HPC * DH))
                # v_aug: per kv tile / head: 64 value dims + a ones
                # column at 64 (denominator lands on psum row 64).
                v_aug = attnP.tile([128, NT, HPC, 80], E4)
                nc.vector.memset(v_aug[:, :, :, DH:80], 0.0)
                nc.vector.memset(v_aug[:, :, :, DH:DH + 1], 1.0)
                qT01 = attnP.tile([128, P], BF)
                qT2 = attnP.tile([128, P], BF)
                kT01 = attnP.tile([128, P], BF)
                kT2 = attnP.tile([128, P], BF)
                at01 = attnP.tile([128, P], BF)
                at2 = attnP.tile([64, P], BF)
                h1_stage = attnP.tile([64, P], BF)

                # ------------ phase 1: LN + transposes + proj + RoPE ------------
                with (
                    tc.tile_pool(name="earlyP", bufs=1) as earlyP,
                    tc.tile_pool(name="lnwk", bufs=2) as lnwk,
                    tc.tile_pool(name="lnslab", bufs=1) as lnslab,
                    tc.tile_pool(name="psProj", bufs=2, space="PSUM") as psProj,
                ):
                    def layer_norm_side(src, dst):
                        # per half: stats for 8 tiles, one batched rsqrt,
                        # then normalize (affine folded into the
                        # downstream projection weights host-side)
                        HB = NT // 4
                        for hb in range(4):
                            xts = lnslab.tile([128, HB, DIM], F32, tag="lnx")
                            mv = lnwk.tile([128, HB, 2], F32, tag="bnmv")
                            for tt in range(HB):
                                t = hb * HB + tt
                                nc.sync.dma_start(
                                    xts[:, tt, :],
                                    src[t * 128:(t + 1) * 128, :])
                                st = lnwk.tile(
                                    [128, 2, nc.vector.BN_STATS_DIM],
                                    F32, tag="bnst")
                                xg = xts[:, tt, :].rearrange(
                                    "p (g d) -> p g d", g=2)
                                for g in range(2):
                                    nc.vector.bn_stats(st[:, g, :],
                                                       xg[:, g, :])
                                nc.vector.bn_aggr(mv[:, tt, :], st[:])
                            rs = lnwk.tile([128, HB], F32, tag="lnrs")
                            nc.scalar.activation(rs[:], mv[:, :, 1], AF.Ln,
                                                 bias=eps_sb[:], scale=1.0)
                            nc.scalar.activation(rs[:], rs[:], AF.Exp,
                                                 bias=0.0, scale=-0.5)
                            nsc = lnwk.tile([128, HB], F32, tag="lnns")
                            nc.vector.tensor_mul(nsc[:], mv[:, :, 0], rs[:])
                            nc.vector.tensor_scalar_mul(nsc[:], nsc[:], -1.0)
                            for tt in range(HB):
                                t = hb * HB + tt
                                xbf = lnwk.tile([128, DIM], BF, tag="lnbf")
                                nc.scalar.activation(xbf[:], xts[:, tt, :],
                                                     AF.Identity,
                                                     bias=nsc[:, tt:tt + 1],
                                                     scale=rs[:, tt:tt + 1])
                                nc.sync.dma_start(
                                    dst[t * 128:(t + 1) * 128, :], xbf[:])

                    def rope_apply4(nc, src_f32, gt4, dst_bf):
                        # src/dst: [128, 4, 192]; gt4: [128, 4, 2, 32].
                        # Walrus caps DVE ops at 3 free axes, so first
                        # materialize merged [g, (a j)] cos/sin tiles,
                        # then every rope op is [p, g, h, 32].
                        cos_m = work.tile([128, 4, 32], F32, tag="cosm")
                        sin_m = work.tile([128, 4, 32], F32, tag="sinm")
                        nc.vector.tensor_copy(cos_m[:].rearrange(
                            "p g (a j) -> p g a j", a=2),
                            gt4[:, :, :, 0:16])
                        nc.vector.tensor_copy(sin_m[:].rearrange(
                            "p g (a j) -> p g a j", a=2),
                            gt4[:, :, :, 16:32])
                        s5 = src_f32[:].rearrange(
                            "p g (h f two) -> p g h f two", h=HPC, two=2)
                        d5 = dst_bf[:].rearrange(
                            "p g (h f two) -> p g h f two", h=HPC, two=2)
                        cos4 = cos_m[:, :, None, :].to_broadcast(
                            (128, 4, HPC, 32))
                        sin4 = sin_m[:, :, None, :].to_broadcast(
                            (128, 4, HPC, 32))
                        xe = s5[:, :, :, :, 0]
                        xo = s5[:, :, :, :, 1]
                        ta = work.tile([128, 4, HPC, 32], F32, tag="rta")
                        tb = work.tile([128, 4, HPC, 32], F32, tag="rtb")
                        nc.vector.tensor_mul(ta[:], xe, cos4)
                        nc.vector.tensor_mul(tb[:], xo, sin4)
                        nc.vector.tensor_tensor(d5[:, :, :, :, 0], ta[:],
                                                tb[:], ALU.subtract)
                        nc.vector.tensor_mul(ta[:], xe, sin4)
                        nc.vector.tensor_mul(tb[:], xo, cos4)
                        nc.vector.tensor_tensor(d5[:, :, :, :, 1], ta[:],
                                                tb[:], ALU.add)

                    def proj_side(nT, is_q):
                        pos_sb = posq_sb if is_q else poskv_sb
                        rot_dram = qrot_dram if is_q else krot_dram
                        for tg in range(NT // 4):
                            gt4 = work.tile([128, 4, 2, 32], F32,
                                            tag="gq" if is_q else "gk")
                            for tt in range(4):
                                t = tg * 4 + tt
                                for a in range(2):
                                    nc.gpsimd.indirect_dma_start(
                                        out=gt4[:, tt, a, :], out_offset=None,
                                        in_=trig[:],
                                        in_offset=bass.IndirectOffsetOnAxis(
                                            ap=pos_sb[:, t, a:a + 1], axis=0))
                            qb4 = work.tile([128, 4, HPC * DH], F32,
                                            tag="pb4")
                            for tt in range(4):
                                t = tg * 4 + tt
                                if is_q:
                                    q_ps = psProj.tile([128, HPC * DH], F32,
                                                       tag="qps")
                                    for k in range(KT):
                                        nc.tensor.matmul(
                                            q_ps[:],
                                            nT[:, k, t * 128:(t + 1) * 128],
                                            wq_sb[:, k, :], start=(k == 0),
                                            stop=(k == KT - 1))
                                    nc.vector.tensor_add(
                                        qb4[:, tt, :], q_ps[:],
                                        bqkv_rep[:, 0:HPC * DH])
                                else:
                                    kv_ps = psProj.tile(
                                        [128, 2 * HPC * DH], F32, tag="kvps")
                                    for k in range(KT):
                                        nc.tensor.matmul(
                                            kv_ps[:],
                                            nT[:, k, t * 128:(t + 1) * 128],
                                            wkv_sb[:, k, :], start=(k == 0),
                                            stop=(k == KT - 1))
                                    nc.vector.tensor_add(
                                        qb4[:, tt, :], kv_ps[:, 0:HPC * DH],
                                        bqkv_rep[:, HPC * DH:2 * HPC * DH])
                                    nc.vector.tensor_tensor(
                                        v_aug[:, t, :, 0:DH],
                                        kv_ps[:,
                                              HPC * DH:2 * HPC * DH].rearrange(
                                            "p (h d) -> p h d", h=HPC),
                                        bqkv_rep[:,
                                                 2 * HPC * DH:3 * HPC * DH
                                                 ].rearrange(
                                            "p (h d) -> p h d", h=HPC),
                                        ALU.add)
                            rot4 = work.tile([128, 4, HPC * DH], BF,
                                             tag="rot4")
                            rope_apply4(nc, qb4, gt4, rot4)
                            r0, r1 = tg * 512, (tg + 1) * 512
                            nc.sync.dma_start(
                                rot_dram[r0:r1, 0:HPC * DH].rearrange(
                                    "(g p) d -> p g d", p=128), rot4[:])
                            # duplicate head 2 into cols 192:256 so the
                            # transposed tile has head 2 on partitions
                            # 64..127 as well (for split-kv scores)
                            nc.sync.dma_start(
                                rot_dram[r0:r1, 192:256].rearrange(
                                    "(g p) d -> p g d", p=128),
                                rot4[:, :, 128:192])

                    # q side, then kv side (shared nT slab, tag reuse)
                    layer_norm_side(query, qn_dram)
                    nT_q = earlyP.tile([128, KT, P], BF, tag="nT", name="nTq")
                    for m in range(KT):
                        nc.sync.dma_start_transpose(
                            nT_q[:, m, :], qn_dram[:, m * 128:(m + 1) * 128])
                    proj_side(nT_q, True)

                    layer_norm_side(kv, kvn_dram)
                    nT_kv = earlyP.tile([128, KT, P], BF, tag="nT", name="nTkv")
                    for m in range(KT):
                        nc.sync.dma_start_transpose(
                            nT_kv[:, m, :], kvn_dram[:, m * 128:(m + 1) * 128])
                    proj_side(nT_kv, False)

                # MLP weights (fp8, x64 scaled host-side): loaded here so
                # the DMA overlaps attention, after the LN slab is freed
                w1_sb = mlpw.tile([128, KT, MLP_H], E4)
                nc.sync.dma_start(w1_sb[:],
                                  w1.rearrange("(k p) n -> p k n", p=128))
                w2_sb = mlpw.tile([128, MLP_H // 128, DIM], E4)
                nc.sync.dma_start(w2_sb[:],
                                  w2.rearrange("(k p) n -> p k n", p=128))

                # Q,K -> head-dim-major (cols 192:256 carry head2 again)
                nc.sync.dma_start_transpose(qT01[:], qrot_dram[:, 0:128])
                nc.sync.dma_start_transpose(qT2[:], qrot_dram[:, 128:256])
                nc.sync.dma_start_transpose(kT01[:], krot_dram[:, 0:128])
                nc.sync.dma_start_transpose(kT2[:], krot_dram[:, 128:256])

                # ------------ phase 3: attention ------------
                scale = DH ** (-0.5)
                with (
                    tc.tile_pool(name="prbP", bufs=2) as prbP,
                    tc.tile_pool(name="psS", bufs=2, space="PSUM") as psS,
                    tc.tile_pool(name="psAttn", bufs=1, space="PSUM") as psA,
                    tc.tile_pool(name="psRep", bufs=2, space="PSUM") as psRep,
                ):
                    def normalize(a_ps, dst_ap):
                        # a_ps rows 0..63 = attn dims, rows 64..127 =
                        # denominator (replicated). One row -> bf16 (same
                        # lane, partition 64), K=1 matmul broadcast back
                        # to rows 0..63, then a single divide.
                        rcp_f = work.tile([128, QC], F32, tag="den")
                        with nc.allow_low_precision(
                                reason="softmax normalize in bf16 matches "
                                       "bf16/fp8 attn matmul precision"):
                            nc.scalar.activation(rcp_f[64:65, :],
                                                 a_ps[64:65, :], AF.Ln)
                            nc.scalar.activation(rcp_f[64:65, :],
                                                 rcp_f[64:65, :], AF.Exp,
                                                 bias=0.0, scale=-1.0)
                            rep_ps = psRep.tile([64, QC], F32, tag="rep")
                            nc.tensor.matmul(rep_ps[:], ones_f32[64:65, 0:64],
                                             rcp_f[64:65, :], start=True,
                                             stop=True)
                            at_sb = work.tile([64, QC], BF, tag="atsb")
                            nc.vector.tensor_copy(at_sb[:], a_ps[0:64, :])
                            # DVE reads at most one PSUM operand: numerator
                            # from SBUF, 1/den replicated in PSUM
                            nc.vector.tensor_mul(dst_ap, at_sb[:], rep_ps[:])

                    # heads 0,1: adjacent matmuls on disjoint row groups
                    for c in range(NQC):
                        a0 = psA.tile([80, QC], F32, tag="a0")
                        a1 = psA.tile([80, QC], F32, tag="a1")
                        for ip in range(NT // 2):
                            prb = prbP.tile([128, 2, 2, QC], E4, tag="prb")
                            for ii in range(2):
                                i = 2 * ip + ii
                                sg = psS.tile([128, 2, QC], F32, tag="sg")
                                nc.tensor.matmul(
                                    sg[:, 0, :],
                                    kT01[0:64, i * 128:(i + 1) * 128],
                                    qT01[0:64, c * QC:(c + 1) * QC],
                                    start=True, stop=True)
                                nc.tensor.matmul(
                                    sg[:, 1, :],
                                    kT01[64:128, i * 128:(i + 1) * 128],
                                    qT01[64:128, c * QC:(c + 1) * QC],
                                    start=True, stop=True)
                                nc.scalar.activation(
                                    prb[:, ii, :, :].rearrange(
                                        "p h n -> p (h n)"),
                                    sg[:].rearrange("p h n -> p (h n)"),
                                    AF.Exp, bias=0.0, scale=scale)
                            for h in range(2):
                                nc.tensor.matmul(
                                    (a0 if h == 0 else a1)[:],
                                    v_aug[:, 2 * ip:2 * ip + 2, h, :],
                                    prb[:, :, h, :],
                                    start=(ip == 0), stop=(ip == NT // 2 - 1),
                                    perf_mode=PM.DoubleRow,
                                    skip_group_check=True)
                        normalize(a0, at01[0:64, c * QC:(c + 1) * QC])
                        normalize(a1, h1_stage[:, c * QC:(c + 1) * QC])
                    nc.sync.dma_start(at01[64:128, :], h1_stage[:])
                    # exchange pair-head attn outputs now; overlaps
                    # head-2. 8-core AllToAll (4-core mesh unsupported):
                    # each quarter goes to blocks j and j+4 masked by the
                    # per-core batch mask, so cross-batch blocks are zero
                    # and the receiver just sums the lo/hi halves.
                    for j in range(G):
                        for hb in range(2):
                            mq = work.tile([128, QC], BF, tag="mq")
                            nc.vector.tensor_scalar_mul(
                                mq[:], at01[:, j * QC:(j + 1) * QC],
                                msk_sb[:, hb:hb + 1])
                            nc.sync.dma_start(ag_a_in[hb * G + j, :, :],
                                              mq[:])
                    nc.gpsimd.collective_compute(
                        "AllToAll", ALU.bypass, replica_groups=GROUPS8,
                        ins=[ag_a_in[:].opt()], outs=[ag_a_out[:].opt()])

                    # head 2: kv tiles i / i+8 on disjoint row groups
                    v2view = v_aug[:].rearrange("p (s t) h d -> p t s h d",
                                                s=2)
                    for c in range(NQC):
                        a2 = psA.tile([80, QC], F32, tag="a0", name=f"a2_{c}")
                        for i in range(NT // 2):
                            sg = psS.tile([128, 2, QC], F32, tag="sg",
                                              name=f"sg2_{c}_{i}")
                            nc.tensor.matmul(
                                sg[:, 0, :],
                                kT2[0:64, i * 128:(i + 1) * 128],
                                qT2[0:64, c * QC:(c + 1) * QC],
                                start=True, stop=True)
                            nc.tensor.matmul(
                                sg[:, 1, :],
                                kT2[64:128, (i + 8) * 128:(i + 9) * 128],
                                qT2[64:128, c * QC:(c + 1) * QC],
                                start=True, stop=True)
                            prb = prbP.tile([128, 2, QC], E4, tag="prb2")
                            nc.scalar.activation(
                                prb[:].rearrange("p s n -> p (s n)"),
                                sg[:].rearrange("p s n -> p (s n)"),
                                AF.Exp, bias=0.0, scale=scale)
                            nc.tensor.matmul(
                                a2[:], v2view[:, i, :, 2, :], prb[:],
                                start=(i == 0), stop=(i == NT // 2 - 1),
                                perf_mode=PM.DoubleRow,
                                skip_group_check=True)
                        normalize(a2, at2[:, c * QC:(c + 1) * QC])
                    for j in range(G):
                        for hb in range(2):
                            mq = work.tile([64, QC], BF, tag="mq2")
                            nc.vector.tensor_scalar_mul(
                                mq[:], at2[:, j * QC:(j + 1) * QC],
                                msk_sb[0:64, hb:hb + 1])
                            nc.sync.dma_start(ag_b_in[hb * G + j, :, :],
                                              mq[:])
                    nc.gpsimd.collective_compute(
                        "AllToAll", ALU.bypass, replica_groups=GROUPS8,
                        ins=[ag_b_in[:].opt()], outs=[ag_b_out[:].opt()])

                # ------------ phase 4: local o-proj + residual ------------
                with (
                    tc.tile_pool(name="oprojP", bufs=1) as oprojP,
                    tc.tile_pool(name="psO", bufs=1, space="PSUM") as psO,
                ):
                    x_sb = xP.tile([128, KT, TPC], F32, name="x_sb")
                    qres_sb = oprojP.tile([128, KT, TPC], F32,
                                          name="qres_sb")
                    nc.sync.dma_start(
                        qres_sb[:],
                        q_res_t[:].rearrange("(m p) n -> p m n", p=128))
                    ata_sb = oprojP.tile([128, G, TPC], BF)
                    ata_hi = oprojP.tile([128, G, TPC], BF)
                    nc.sync.dma_start(
                        ata_sb[:],
                        ag_a_out[0:G, :, :].rearrange("g p n -> p g n"))
                    nc.sync.dma_start(
                        ata_hi[:],
                        ag_a_out[G:2 * G, :, :].rearrange("g p n -> p g n"))
                    nc.vector.tensor_add(ata_sb[:], ata_sb[:], ata_hi[:])
                    xo = [psO.tile([128, TPC], F32, tag=f"xo{m}",
                                   name=f"xo_{m}")
                          for m in range(KT)]
                    for m in range(KT):
                        for k in range(G):
                            nc.tensor.matmul(
                                xo[m][:],
                                woa_sb[:, k, m * 128:(m + 1) * 128],
                                ata_sb[:, k, :], start=(k == 0), stop=False,
                                skip_group_check=True)
                    atb_sb = oprojP.tile([128, 2, TPC], BF)
                    atb_hi = oprojP.tile([128, 2, TPC], BF)
                    nc.sync.dma_start(
                        atb_sb[:],
                        ag_b_out[0:G, :, :].rearrange(
                            "(k two) p n -> (two p) k n", two=2))
                    nc.sync.dma_start(
                        atb_hi[:],
                        ag_b_out[G:2 * G, :, :].rearrange(
                            "(k two) p n -> (two p) k n", two=2))
                    nc.vector.tensor_add(atb_sb[:], atb_sb[:], atb_hi[:])
                    for m in range(KT):
                        for k in range(2):
                            nc.tensor.matmul(
                                xo[m][:],
                                wob_sb[:, k, m * 128:(m + 1) * 128],
                                atb_sb[:, k, :], start=False, stop=(k == 1),
                                skip_group_check=True)
                        nc.vector.tensor_add(x_sb[:, m, :], xo[m][:],
                                             qres_sb[:, m, :])

            # ------------ phase 5: MLP ------------
            with tc.tile_pool(name="mlpP", bufs=1) as mlpP:
                xn_sb = mlpP.tile([128, KT, TPC], E4)
                with (
                    tc.tile_pool(name="mlptmp", bufs=1) as mlptmp,
                    tc.tile_pool(name="psStat", bufs=1, space="PSUM") as psStat,
                ):
                    xb = mlptmp.tile([128, KT, TPC], BF)
                    sqb = mlptmp.tile([128, KT, TPC], BF)
                    mean_ps = psStat.tile([1, TPC], F32, tag="meanps")
                    sq_ps = psStat.tile([1, TPC], F32, tag="sqps")
                    for m in range(KT):
                        nc.scalar.activation(xb[:, m, :], x_sb[:, m, :],
                                             AF.Identity)
                        nc.scalar.activation(sqb[:, m, :], x_sb[:, m, :],
                                             AF.Square)
                        nc.tensor.matmul(mean_ps[:], ones_bf[:, 0:1],
                                         xb[:, m, :], start=(m == 0),
                                         stop=(m == KT - 1),
                                         skip_group_check=True)
                        nc.tensor.matmul(sq_ps[:], ones_bf[:, 0:1],
                                         sqb[:, m, :], start=(m == 0),
                                         stop=(m == KT - 1),
                                         skip_group_check=True)
                    # mean, std rows -> bf16, broadcast via K=1 matmul
                    mrow_bf = mlptmp.tile([1, TPC], BF)
                    vrow = mlptmp.tile([1, TPC], F32)
                    msq = mlptmp.tile([1, TPC], F32)
                    nc.scalar.activation(mrow_bf[:], mean_ps[:], AF.Identity,
                                         bias=0.0, scale=1.0 / DIM)
                    nc.vector.tensor_scalar_mul(vrow[:], sq_ps[:], 1.0 / DIM)
                    nc.vector.tensor_mul(msq[:], mrow_bf[:], mrow_bf[:])
                    nc.vector.tensor_tensor(vrow[:], vrow[:], msq[:],
                                            ALU.subtract)
                    rrow_f = mlptmp.tile([1, TPC], F32)
                    nc.scalar.activation(rrow_f[:], vrow[:], AF.Ln,
                                         bias=eps_sb[0:1, :], scale=1.0)
                    nc.scalar.activation(rrow_f[:], rrow_f[:], AF.Exp,
                                         bias=0.0, scale=-0.5)
                    with tc.tile_pool(name="psReps", bufs=1,
                                      space="PSUM") as psReps:
                        mrep_ps = psReps.tile([128, TPC], F32, tag="mrep")
                        nc.tensor.matmul(mrep_ps[:], ones_bf[0:1, :],
                                         mrow_bf[:], start=True, stop=True)
                        srep_ps = psReps.tile([128, TPC], F32, tag="srep")
                        nc.tensor.matmul(srep_ps[:], ones_f32[0:1, :],
                                         rrow_f[:], start=True, stop=True)
                        for m in range(KT):
                            t1 = work.tile([128, TPC], F32, tag="mlnt1")
                            nc.vector.tensor_tensor(
                                t1[:], x_sb[:, m, :], mrep_ps[:],
                                ALU.subtract)
                            with nc.allow_low_precision(
                                    reason="LN normalize feeding fp8 matmul"):
                                nc.vector.tensor_mul(
                                    xn_sb[:, m, :], t1[:], srep_ps[:])

                # FF1 + GELU (fp8 DoubleRow; weights x64, so gelu scale 1/64)
                h_sb = mlpP.tile([128, MLP_H // 128, TPC], E4)
                with tc.tile_pool(name="psF1", bufs=1, space="PSUM") as psF1:
                    for jp in range(MLP_H // 256):
                        f1 = psF1.tile([128, 2, TPC], F32, tag="f1")
                        for jj in range(2):
                            j = jp * 2 + jj
                            for kk in range(KT // 2):
                                nc.tensor.matmul(
                                    f1[:, jj, :],
                                    w1_sb[:, 2 * kk:2 * kk + 2,
                                          j * 128:(j + 1) * 128],
                                    xn_sb[:, 2 * kk:2 * kk + 2, :],
                                    start=(kk == 0), stop=(kk == KT // 2 - 1),
                                    perf_mode=PM.DoubleRow,
                                    skip_group_check=True)
                            nc.scalar.activation(
                                h_sb[:, j, :], f1[:, jj, :], AF.Gelu,
                                bias=b1_sb[:, j:j + 1], scale=1.0 / W8)

                    # FF2 + residual + out
                    with tc.tile_pool(name="psF2", bufs=1,
                                      space="PSUM") as psF2:
                        f2 = [psF2.tile([128, TPC], F32, tag=f"f2_{m}",
                                        name=f"f2t_{m}")
                              for m in range(KT)]
                        for jj in range(MLP_H // 256):
                            for m in range(KT):
                                nc.tensor.matmul(
                                    f2[m][:],
                                    w2_sb[:, 2 * jj:2 * jj + 2,
                                          m * 128:(m + 1) * 128],
                                    h_sb[:, 2 * jj:2 * jj + 2, :],
                                    start=(jj == 0),
                                    stop=(jj == MLP_H // 256 - 1),
                                    perf_mode=PM.DoubleRow,
                                    skip_group_check=True)
                        for m in range(KT):
                            fo = work.tile([128, TPC], F32, tag="fo")
                            nc.scalar.activation(fo[:], f2[m][:], AF.Identity,
                                                 bias=b2_sb[:, m:m + 1],
                                                 scale=1.0 / W8)
                            nc.vector.tensor_add(fo[:], fo[:], x_sb[:, m, :])
                            nc.sync.dma_start(
                                out_t[m * 128:(m + 1) * 128, :], fo[:])

    _split_multi_waits(nc)
    return nc


_NC_CACHE = None


def _get_nc():
    global _NC_CACHE
    if _NC_CACHE is None:
        _NC_CACHE = build_nc()
    return _NC_CACHE


def _make_trig():
    j = np.arange(16)
    f = 1.0 / (ROPE_THETA ** (2.0 * j / 32.0))
    v = np.arange(64)
    ang = v[:, None] * f[None, :]
    return np.concatenate([np.cos(ang), np.sin(ang)], axis=1).astype(np.float32)


def kernel(**inputs):
    from concourse.bass_utils import run_bass_kernel_spmd

    np32 = lambda x: np.asarray(x, dtype=np.float32)
    npbf = lambda x: np.asarray(np.asarray(x, dtype=np.float32),
                                dtype=ml_dtypes.bfloat16)

    def npe4(x):
        return np.asarray(
            np.clip(np.asarray(x, dtype=np.float32), -240.0, 240.0),
            dtype=ml_dtypes.float8_e4m3)

    query = np32(inputs["query"])
    kv = np32(inputs["kv"])
    pos_q = np.asarray(inputs["pos_q"]).astype(np.int32)
    pos_kv = np.asarray(inputs["pos_kv"]).astype(np.int32)

    # fold the LN affine transforms into the downstream projections
    lnqw, lnqb = np32(inputs["ln_q_w"]), np32(inputs["ln_q_b"])
    lnkw, lnkb = np32(inputs["ln_kv_w"]), np32(inputs["ln_kv_b"])
    lnmw, lnmb = np32(inputs["ln_mlp_w"]), np32(inputs["ln_mlp_b"])
    wq_f = lnqw[:, None] * np32(inputs["wq"])
    bq_f = np32(inputs["bq"]) + lnqb @ np32(inputs["wq"])
    wk_f = lnkw[:, None] * np32(inputs["wk"])
    bk_f = np32(inputs["bk"]) + lnkb @ np32(inputs["wk"])
    wv_f = lnkw[:, None] * np32(inputs["wv"])
    bv_f = np32(inputs["bv"]) + lnkb @ np32(inputs["wv"])
    w1_f = lnmw[:, None] * np32(inputs["w1"])
    b1_f = np32(inputs["b1"]) + lnmb @ np32(inputs["w1"])

    wq, wk, wv = npbf(wq_f), npbf(wk_f), npbf(wv_f)
    wo = npbf(inputs["wo"])
    wo_h = np.asarray(wo).reshape(H, DH, DIM)
    wo_a = np.ascontiguousarray(
        wo_h[[0, 1, 3, 4, 6, 7, 9, 10]].reshape(G * 128, DIM))
    wo_b = np.ascontiguousarray(
        wo_h[[2, 5, 8, 11]].reshape(G * 64, DIM))
    w1_8 = npe4(w1_f * W8)
    w2_8 = npe4(np32(inputs["w2"]) * W8)
    bo = np32(inputs["bo"])
    trig = _make_trig()

    in_maps = []
    for c in range(N_CORES):
        b, s = c // G, c % G
        hs = slice(HPC * DH * s, HPC * DH * (s + 1))
        ts = slice(TPC * s, TPC * (s + 1))
        mskv = np.zeros((128, 2), np.float32)
        mskv[:, 0 if b == 0 else 1] = 1.0
        in_maps.append({
            "query": query[b],
            "kv": kv[b],
            "msk": mskv,
            # residual with the o-proj bias folded in
            "q_res_t": np.ascontiguousarray(
                (query[b, ts, :] + bo[None, :]).T),
            "posq": np.ascontiguousarray(
                pos_q[b].reshape(NT, 128, 2).transpose(1, 0, 2)),
            "poskv": np.ascontiguousarray(
                pos_kv[b].reshape(NT, 128, 2).transpose(1, 0, 2)),
            "trig": trig,
            "wq_s": np.ascontiguousarray(wq[:, hs]),
            "wkv_s": np.ascontiguousarray(
                np.concatenate([wk[:, hs], wv[:, hs]], axis=1)),
            "bqkv_s": np.concatenate(
                [bq_f[hs], bk_f[hs], bv_f[hs]]).astype(np.float32),
            "wo_a": wo_a, "wo_b": wo_b,
            "w1": w1_8, "b1": b1_f.astype(np.float32),
            "w2": w2_8, "b2": np32(inputs["b2"]),
        })

    nc = _get_nc()
    res = run_bass_kernel_spmd(nc, in_maps, core_ids=list(range(N_CORES)))

    out = np.empty((B, P, DIM), np.float32)
    for c in range(N_CORES):
        b, s = c // G, c % G
        out[b, TPC * s:TPC * (s + 1), :] = res.results[c]["out_t"].T
    return out
